# revision 32
# baseline (speedup 1.0000x reference)
"""Bass/Trainium2 kernel for attention-LSTM decoder (nn_Attention_49289044688898).

Data-parallel over batch: 512 rows -> 8 NeuronCores x 64 rows. Weights replicated.
Within a core, 64 rows = 2 groups of 32 for the attention; LSTM/q/probs joint.

v2 schedule (vs v1): no DMA transposes (PE transposes + direct-transposed qT
matmul), oh/h@R/probs matmuls hoisted into the tanh window, exp before the
e-scatter DRAM roundtrip with softmax normalization folded into the ctx
PSUM->SBUF copy, AF.Sigmoid for gates.

Per step s (26 steps):
  hT  = transpose(h)                        (PE, 4 transpose-mm)
  qT  = WhT-chunks @ hT                     (PE, 16 mm N=64, k-accum)
  probs(s-1) = hT-mm @ Wgen + bg            (PE + DVE, during tanh window)
  z-partial: onehot@Ko' + h@R               (PE, during tanh window)
  per group g: th = tanh(HprojT + qT)       (DVE add + ACT tanh, 4 chunks)
               e  = ws-quadrant mms         (PE)
               ex = exp(e) (PSUM->SBUF est) (ACT, no max-sub; e is bounded)
               alphaT[t,b] built from est rows by 4 row-spread SBUF DMAs
                 (est row 32j == alphaT rows 16j..16j+16; gpsimd+sync queues)
               ablk block-diag scatter from alphaT (2 DMA, sync)
               denominator = alphaT^T @ ones (1 matmul) -> reciprocal (DVE)
               ctx = ablk@bHc mms; scale by 1/sum in PSUM->SBUF copy
  xTc = PE-transpose(ctx)
  z  += xTc @ Kc                            (PE)
  gates: sigmoid/tanh (ACT) + c/h (DVE)
Layouts:
  attention world: [128 part = h_lo, 4 h_hi, 64 t, 32 b]
  context world:   [128 part = (b%2)*64 + t, 16 kt=b//2, 512 c]
  LSTM world:      [64 part = b, free]
"""

import numpy as np
import ml_dtypes
from contextlib import ExitStack

B, T, C, H, NCC, S = 512, 64, 512, 512, 96, 26
NCORES = 8
BS = B // NCORES          # 64 batch rows per core
NG = 2                    # groups per core
GB = BS // NG             # 32 rows per group
BF = ml_dtypes.bfloat16

_CACHE = {}


def build_bass():
    import concourse.bass as bass
    import concourse.bacc as bacc
    import concourse.tile as tile
    import concourse.mybir as mybir

    f32 = mybir.dt.float32
    bf16 = mybir.dt.bfloat16
    AF = mybir.ActivationFunctionType
    AX = mybir.AxisListType

    nc = bacc.Bacc("TRN2", target_bir_lowering=False)

    # ---- DRAM I/O ----
    bHT_d = nc.dram_tensor("bHT", [NG, C, T, GB], bf16, kind="ExternalInput")
    bHc_d = nc.dram_tensor("bHc", [NG, GB // 2, 128, C], bf16, kind="ExternalInput")
    wi_d = nc.dram_tensor("wi", [C, H], bf16, kind="ExternalInput")
    wh_d = nc.dram_tensor("wh", [H, H], bf16, kind="ExternalInput")
    bh_d = nc.dram_tensor("bh", [128, 4], f32, kind="ExternalInput")
    ws_d = nc.dram_tensor("ws", [128, 4, 32], bf16, kind="ExternalInput")
    kc_d = nc.dram_tensor("kc", [C, 4 * H], bf16, kind="ExternalInput")
    rr_d = nc.dram_tensor("rr", [H, 4 * H], bf16, kind="ExternalInput")
    ko_d = nc.dram_tensor("ko", [NCC, 4 * H], bf16, kind="ExternalInput")
    oh_d = nc.dram_tensor("oh", [NCC, S, BS], bf16, kind="ExternalInput")
    wg_d = nc.dram_tensor("wg", [H, NCC], bf16, kind="ExternalInput")
    bg_d = nc.dram_tensor("bg", [BS, NCC], f32, kind="ExternalInput")
    id_d = nc.dram_tensor("ident", [128, 128], bf16, kind="ExternalInput")
    out_d = nc.dram_tensor("out", [BS, S, NCC], f32, kind="ExternalOutput")

    NCH = T * GB // 512  # 4 (t,b)-chunks of 512 per group

    with tile.TileContext(nc) as tc, ExitStack() as ctx:
        big = ctx.enter_context(tc.tile_pool(name="big", bufs=1))
        wpool = ctx.enter_context(tc.tile_pool(name="wpool", bufs=1))
        small = ctx.enter_context(tc.tile_pool(name="small", bufs=2))
        tiny = ctx.enter_context(tc.tile_pool(name="tiny", bufs=4))
        gates = ctx.enter_context(tc.tile_pool(name="gates", bufs=4))
        state = ctx.enter_context(tc.tile_pool(name="state", bufs=2))
        # PSUM pools (8 banks total):
        #   pz:  FI + GO gate accumulators  [128,512] x2     = 2 banks
        #   pep: e quadrant accumulator     [128,512] bufs=2 = 2 banks
        #   pcp: ctx accumulator            [32,512]  bufs=1 = 1 bank
        #   ptp: bf16 PE-transpose outs     [128,256] bufs=1 = 1 bank
        #   psm: qT/probs/sums f32 mm outs  [128,256] bufs=2 = 2 banks
        pz = ctx.enter_context(tc.tile_pool(name="pz", bufs=1, space="PSUM"))
        pep = ctx.enter_context(tc.tile_pool(name="pep", bufs=2, space="PSUM"))
        pcp = ctx.enter_context(tc.tile_pool(name="pcp", bufs=1, space="PSUM"))
        ptp = ctx.enter_context(tc.tile_pool(name="ptp", bufs=1, space="PSUM"))
        psm = ctx.enter_context(tc.tile_pool(name="psm", bufs=2, space="PSUM"))

        dma = nc.sync
        import concourse.bass as _b

        # ---- load weights / big tensors ----
        bHc = [big.tile([128, GB // 2, C], bf16, tag=f"bHc{g}", name=f"bHc{g}")
               for g in range(NG)]
        for g in range(NG):
            dma.dma_start(out=bHc[g], in_=bHc_d[g].rearrange("k p c -> p k c"))
        # batch_H^T (prolog only; shares slots with tanh buffers)
        bHT = [big.tile([128, 4, T * GB], bf16, tag=f"th{g}", name=f"bHT{g}")
               for g in range(NG)]
        for g in range(NG):
            dma.dma_start(
                out=bHT[g],
                in_=bHT_d[g].rearrange("(ch cl) t b -> cl ch (t b)", cl=128))

        wi = wpool.tile([128, 4, H], bf16, tag="wi")
        dma.dma_start(out=wi, in_=wi_d[:].rearrange("(ch cl) h -> cl ch h", cl=128))
        wh = wpool.tile([128, 4, H], bf16, tag="wh")
        dma.dma_start(out=wh, in_=wh_d[:].rearrange("(hh hl) h -> hl hh h", hl=128))
        bh = wpool.tile([128, 4], f32, tag="bh")
        dma.dma_start(out=bh, in_=bh_d[:])
        ws = wpool.tile([128, 4, 32], bf16, tag="ws")
        dma.dma_start(out=ws, in_=ws_d[:])
        kc = wpool.tile([128, 4, 4 * H], bf16, tag="kc")
        dma.dma_start(out=kc, in_=kc_d[:].rearrange("(kh kl) n -> kl kh n", kl=128))
        rr = wpool.tile([128, 4, 4 * H], bf16, tag="rr")
        dma.dma_start(out=rr, in_=rr_d[:].rearrange("(kh kl) n -> kl kh n", kl=128))
        ko = wpool.tile([NCC, 4 * H], bf16, tag="ko")
        dma.dma_start(out=ko, in_=ko_d[:])
        oh = wpool.tile([NCC, S, BS], bf16, tag="oh")
        dma.dma_start(out=oh, in_=oh_d[:])
        wg = wpool.tile([128, 4, NCC], bf16, tag="wg")
        dma.dma_start(out=wg, in_=wg_d[:].rearrange("(hh hl) n -> hl hh n", hl=128))
        bg = wpool.tile([BS, NCC], f32, tag="bg")
        dma.dma_start(out=bg, in_=bg_d[:])
        ident = wpool.tile([128, 128], bf16, tag="ident")
        dma.dma_start(out=ident, in_=id_d[:])
        ones = wpool.tile([T, 1], bf16, tag="ones")
        nc.vector.memset(ones, 1.0)
        pr_all = wpool.tile([BS, S, NCC], f32, tag="pr_all")

        # block-diag alpha holders (zeroed once)
        ablk = [wpool.tile([128, GB // 2, GB], bf16, tag=f"ablk{g}", name=f"ablk{g}")
                for g in range(NG)]
        for g in range(NG):
            nc.vector.memset(ablk[g], 0.0)

        # initial state
        hT = [state.tile([128, 4, BS], bf16, tag="hT", name="hT0")]
        nc.vector.memset(hT[0], 0.0)
        c_st = [state.tile([BS, H], f32, tag="c", name="c0")]
        nc.vector.memset(c_st[0], 0.0)
        hbf = [None]

        # ---- prolog: HprojT[g] = (batch_H @ Wi)^T + bh ----
        hprojT = [big.tile([128, 4, T * GB], bf16, tag=f"hp{g}", name=f"hp{g}")
                  for g in range(NG)]
        for g in range(NG):
            for m in range(4):
                for n in range(NCH):
                    ps = pz.tile([128, 512], f32, tag="FI" if g == 0 else "GO")
                    for k in range(4):
                        nc.tensor.matmul(
                            ps,
                            wi[:, k, m * 128:(m + 1) * 128],
                            bHT[g][:, k, n * 512:(n + 1) * 512],
                            start=(k == 0), stop=(k == 3),
                        )
                    nc.scalar.activation(
                        out=hprojT[g][:, m, n * 512:(n + 1) * 512], in_=ps,
                        func=AF.Identity, bias=bh[:, m:m + 1], scale=1.0,
                    )

        def bcast_t(ap2):
            # [128, GB(b)] -> [128, T(t, stride0), GB(b)]
            return _b.AP(tensor=ap2.tensor, offset=ap2.offset,
                         ap=[ap2.ap[0], [0, T], ap2.ap[1]])

        gate_sl = {"f": 1, "i": 0, "g": 2, "o": 3}
        # gate -> (psum tag, row offset): f/i share FI bank, g/o share GO bank
        gate_loc = {"f": ("FI", 0), "i": ("FI", 64), "g": ("GO", 0), "o": ("GO", 64)}

        def emit_hT_transpose(s):
            # h_bf [64, 512] -> hT [128, 4, 64] via 4 PE transposes
            phT = ptp.tile([128, 256], bf16, tag="tp", name=f"phT_{s}")
            for m in range(4):
                nc.tensor.transpose(phT[:, m * 64:(m + 1) * 64],
                                    hbf[0][:, m * 128:(m + 1) * 128],
                                    ident[0:BS, 0:BS])
            hT[0] = state.tile([128, 4, BS], bf16, tag="hT", name=f"hT_{s}")
            nc.vector.tensor_copy(hT[0], phT)

        def emit_qT(s):
            # qT[h',b] = sum_h Wh[h,h'] hT[h,b]; m-outer so chunk m is
            # copied out as soon as its k-accumulation finishes.
            pqT = psm.tile([128, 256], f32, tag="pq", name=f"pqT_{s}")
            qT = small.tile([128, 4, BS], bf16, tag="qT", bufs=2, name=f"qT_{s}")
            for m in range(4):
                for k in range(4):
                    nc.tensor.matmul(pqT[:, m * 64:(m + 1) * 64],
                                     wh[:, k, m * 128:(m + 1) * 128],
                                     hT[0][:, k, :],
                                     start=(k == 0), stop=(k == 3))
                nc.vector.tensor_copy(qT[:, m, :], pqT[:, m * 64:(m + 1) * 64])
            return qT

        def emit_probs(sm1):
            # probs(sm1) = h(sm1) @ Wgen + bg, from hT
            pp = psm.tile([128, 256], f32, tag="pq", name=f"pp_{sm1}")
            for k in range(4):
                nc.tensor.matmul(pp[0:BS, 0:NCC], hT[0][:, k, :], wg[:, k, :],
                                 start=(k == 0), stop=(k == 3))
            nc.vector.tensor_add(pr_all[:, sm1, :], pp[0:BS, 0:NCC], bg)

        def emit_z_early(s, pzt):
            # onehot@Ko' (start) + h@R during the tanh window
            for gn in "figo":
                tag, ro = gate_loc[gn]
                zsl = slice(gate_sl[gn] * 512, (gate_sl[gn] + 1) * 512)
                nc.tensor.matmul(pzt[tag][ro:ro + 64, :], oh[:, s, :],
                                 ko[:, zsl], start=True, stop=False,
                                 tile_position=(0, ro))
            for k in range(4):
                for gn in "figo":
                    tag, ro = gate_loc[gn]
                    zsl = slice(gate_sl[gn] * 512, (gate_sl[gn] + 1) * 512)
                    nc.tensor.matmul(pzt[tag][ro:ro + 64, :], hT[0][:, k, :],
                                     rr[:, k, zsl], start=False, stop=False,
                                     tile_position=(0, ro))

        def emit_att_tanh(s, g, qT, pe_):
            # DVE add + ACT tanh + e quadrant mms for group g
            gsl_b = slice(g * GB, (g + 1) * GB)
            th = big.tile([128, 4, T * GB], bf16, tag=f"th{g}", name=f"th{g}_{s}")
            for k in range(4):
                nc.vector.tensor_add(
                    th[:, k, :].rearrange("p (t b) -> p t b", t=T),
                    hprojT[g][:, k, :].rearrange("p (t b) -> p t b", t=T),
                    bcast_t(qT[:, k, gsl_b]))
                nc.scalar.activation(out=th[:, k, :], in_=th[:, k, :], func=AF.Tanh)
                for j in range(NCH):
                    bp = 32 * j
                    nc.tensor.matmul(pe_[bp:bp + 32, :], ws[:, k, :],
                                     th[:, k, j * 512:(j + 1) * 512],
                                     start=(k == 0), stop=(k == 3),
                                     tile_position=(0, bp))

        def emit_exp_scatter(s, g, pe_):
            # exp on the PSUM layout: est[32j, tl*32+b] = ex(t=16j+tl, b).
            # ablk (block-diag) is written DIRECTLY from est (2 DMAs, one
            # per b-parity); alphaT (only feeds the denominator matmul)
            # via one merged DMA on the gpsimd queue.
            est = small.tile([128, 512], bf16, tag=f"est{g}", bufs=1,
                             name=f"est{g}_{s}")
            nc.scalar.activation(out=est, in_=pe_, func=AF.Exp)
            ea = est[:]
            pp = ea.ap[0][0]
            alphaT = small.tile([T, GB], bf16, tag=f"alphaT{g}", bufs=2,
                                name=f"alphaT{g}_{s}")
            at = alphaT[:]
            for j in range(4):
                esl = est[32 * j:32 * j + 1, :]
                srcj = _b.AP(tensor=esl.tensor, offset=esl.offset,
                             ap=[[esl.ap[0][0], 1], [GB, T // 4], [1, GB]])
                eng = nc.gpsimd if j % 2 == 0 else dma
                eng.dma_start(out=alphaT[16 * j:16 * (j + 1), :], in_=srcj)
            # ablk block-diag scatter (2 DMAs, sync queue)
            ab = ablk[g][:]
            for par in (0, 1):
                srcp = _b.AP(tensor=at.tensor, offset=at.offset + par * at.ap[1][0],
                             ap=[[at.ap[0][0], T], [2 * at.ap[1][0], GB // 2]])
                dst = _b.AP(tensor=ab.tensor,
                            offset=ab.offset + par * (64 * ab.ap[0][0] + ab.ap[2][0]),
                            ap=[[ab.ap[0][0], T], [ab.ap[1][0] + 2 * ab.ap[2][0], GB // 2]])
                dma.dma_start(out=dst, in_=srcp)
            return alphaT

        def emit_post(s, g, ctx_sb, alphaT):
            # ctx_sb: per-group [GB, C] tile (base partition 0)
            # denominator: sums[b] = alphaT^T @ ones  (one matmul, N=1)
            psums = psm.tile([128, 256], f32, tag="pq", name=f"psm{g}_{s}")
            nc.tensor.matmul(psums[0:GB, 0:1], alphaT, ones,
                             start=True, stop=True)
            rcp = tiny.tile([GB, 1], f32, tag=f"rcp{g}")
            nc.vector.reciprocal(rcp, psums[0:GB, 0:1])
            pctx = pcp.tile([128, C], f32, tag="ctx", name=f"pctx{g}_{s}")
            for kt in range(GB // 2):
                nc.tensor.matmul(pctx[0:GB, :], ablk[g][:, kt, :],
                                 bHc[g][:, kt, :],
                                 start=(kt == 0), stop=(kt == GB // 2 - 1))
            nc.vector.tensor_scalar_mul(ctx_sb, pctx[0:GB, :], rcp)

        def emit_ctxT(s, g, ctx_sb, pxT, xTc):
            # ctx rows of group g -> xTc[:, k, g*32:(g+1)*32]
            for k in range(4):
                nc.tensor.transpose(pxT[g][:, k * GB:(k + 1) * GB],
                                    ctx_sb[:, k * 128:(k + 1) * 128],
                                    ident[0:GB, 0:GB])
            src = pxT[g][:, 0:128].rearrange("p (k b) -> p k b", k=4)
            nc.vector.tensor_copy(xTc[:, :, g * GB:(g + 1) * GB], src)

        def emit_z_late(s, pzt, xTc):
            for pair in ("fi", "go"):
                for k in range(4):
                    for gn in pair:
                        tag, ro = gate_loc[gn]
                        zsl = slice(gate_sl[gn] * 512, (gate_sl[gn] + 1) * 512)
                        nc.tensor.matmul(pzt[tag][ro:ro + 64, :], xTc[:, k, :],
                                         kc[:, k, zsl], start=False,
                                         stop=(k == 3), tile_position=(0, ro))

        def emit_gates(s, pzt):
            sig = {}
            t1 = t2 = None
            for gn in ("f", "i", "g", "o"):
                tag, ro = gate_loc[gn]
                g_sb = gates.tile([BS, H], f32, tag="gate", bufs=4)
                if gn == "g":
                    nc.scalar.activation(out=g_sb, in_=pzt[tag][ro:ro + 64, :],
                                         func=AF.Tanh)
                else:
                    # sigmoid via tanh: keeps ACT on the exp_and_others table
                    nc.scalar.activation(out=g_sb, in_=pzt[tag][ro:ro + 64, :],
                                         func=AF.Tanh, scale=0.5)
                    nc.vector.tensor_scalar(out=g_sb, in0=g_sb,
                                            scalar1=0.5, scalar2=0.5,
                                            op0=mybir.AluOpType.mult,
                                            op1=mybir.AluOpType.add)
                sig[gn] = g_sb
                if gn == "f":
                    t1 = gates.tile([BS, H], f32, tag="tmp", bufs=2)
                    nc.vector.tensor_mul(t1, sig["f"], c_st[0])
                elif gn == "g":
                    t2 = gates.tile([BS, H], f32, tag="tmp", bufs=2)
                    nc.vector.tensor_mul(t2, sig["i"], sig["g"])
                    c_st[0] = state.tile([BS, H], f32, tag="c", name=f"c_{s}")
                    nc.vector.tensor_add(c_st[0], t1, t2)
            tc_sb = gates.tile([BS, H], f32, tag="tmp", bufs=2)
            nc.scalar.activation(out=tc_sb, in_=c_st[0], func=AF.Tanh)
            hbf[0] = small.tile([BS, H], bf16, tag="h_bf", bufs=2,
                                name=f"h_bf_{s}")
            nc.vector.tensor_mul(hbf[0], sig["o"], tc_sb)

        for s in range(S):
            if s > 0:
                emit_hT_transpose(s)
            qT = emit_qT(s)
            if s > 0:
                emit_probs(s - 1)
            pzt = {"FI": pz.tile([128, 512], f32, tag="FI", name=f"pzFI_{s}"),
                   "GO": pz.tile([128, 512], f32, tag="GO", name=f"pzGO_{s}")}
            emit_z_early(s, pzt)
            pe_ = [None] * NG
            ctx_sb = [small.tile([GB, C], bf16, tag=f"ctx_sb{g}", bufs=2,
                                 name=f"ctx{g}_{s}") for g in range(NG)]
            pxT = [None] * NG
            xTc = small.tile([128, 4, BS], bf16, tag="xTc", bufs=2,
                             name=f"xTc_{s}")
            aTs = [None] * NG
            for g in range(NG):
                pe_[g] = pep.tile([128, 512], f32, tag="pe", name=f"pe{g}_{s}")
                emit_att_tanh(s, g, qT, pe_[g])
                aTs[g] = emit_exp_scatter(s, g, pe_[g])
            for g in range(NG):
                emit_post(s, g, ctx_sb[g], aTs[g])
                pxT[g] = ptp.tile([128, 256], bf16, tag="tp", name=f"pxT{g}_{s}")
                emit_ctxT(s, g, ctx_sb[g], pxT, xTc)
            emit_z_late(s, pzt, xTc)
            emit_gates(s, pzt)
        emit_hT_transpose(S)
        emit_probs(S - 1)
        dma.dma_start(out=out_d[:], in_=pr_all)

    nc.finalize()
    return nc


def _prep_core(inputs, i):
    bsl = slice(i * BS, (i + 1) * BS)
    bh_i = np.asarray(inputs["batch_H"][bsl], np.float32)          # [64, 64, 512]
    text_i = np.asarray(inputs["text"][bsl])                       # [64, 26]
    bh_g = bh_i.reshape(NG, GB, T, C)
    m = {}
    m["bHT"] = np.ascontiguousarray(bh_g.transpose(0, 3, 2, 1)).astype(BF)
    m["bHc"] = np.ascontiguousarray(bh_g.reshape(NG, GB // 2, 128, C)).astype(BF)
    m["wi"] = np.asarray(inputs["Wi"], np.float32).astype(BF)
    m["wh"] = np.asarray(inputs["Wh"], np.float32).astype(BF)
    m["bh"] = np.ascontiguousarray(
        np.asarray(inputs["bh"], np.float32).reshape(4, 128).T)
    wsr = np.ascontiguousarray(
        np.asarray(inputs["Ws"], np.float32)[:, 0].reshape(4, 128).T).astype(BF)
    m["ws"] = np.repeat(wsr[:, :, None], 32, axis=2)
    lk = np.asarray(inputs["lstm_kernel"], np.float32)
    lb = np.asarray(inputs["lstm_bias"], np.float32)
    m["kc"] = lk[:C].astype(BF)
    m["ko"] = (lk[C:] + lb[None, :]).astype(BF)
    m["rr"] = np.asarray(inputs["lstm_rec"], np.float32).astype(BF)
    m["oh"] = (np.arange(NCC)[:, None, None] == text_i.T[None, :, :]).astype(BF)
    m["wg"] = np.asarray(inputs["Wgen"], np.float32).astype(BF)
    m["bg"] = np.tile(np.asarray(inputs["bgen"], np.float32)[None, :], (BS, 1))
    m["ident"] = np.eye(128, dtype=np.float32).astype(BF)
    return m


def kernel(_trace=False, **inputs):
    from concourse import bass_utils
    if "nc" not in _CACHE:
        _CACHE["nc"] = build_bass()
    nc = _CACHE["nc"]
    in_maps = [_prep_core(inputs, i) for i in range(NCORES)]
    res = bass_utils.run_bass_kernel_spmd(nc, in_maps, list(range(NCORES)),
                                          trace=_trace)
    _CACHE["last_result"] = res
    out = np.concatenate([r["out"] for r in res.results], axis=0)
    return out.astype(np.float32)


# revision 33
# speedup vs baseline: 1.0019x; 1.0019x over previous
"""Bass/Trainium2 kernel for attention-LSTM decoder (nn_Attention_49289044688898).

Data-parallel over batch: 512 rows -> 8 NeuronCores x 64 rows. Weights replicated.
Within a core, 64 rows = 2 groups of 32 for the attention; LSTM/q/probs joint.

v2 schedule (vs v1): no DMA transposes (PE transposes + direct-transposed qT
matmul), oh/h@R/probs matmuls hoisted into the tanh window, exp before the
e-scatter DRAM roundtrip with softmax normalization folded into the ctx
PSUM->SBUF copy, AF.Sigmoid for gates.

Per step s (26 steps):
  hT  = transpose(h)                        (PE, 4 transpose-mm)
  qT  = WhT-chunks @ hT                     (PE, 16 mm N=64, k-accum)
  probs(s-1) = hT-mm @ Wgen + bg            (PE + DVE, during tanh window)
  z-partial: onehot@Ko' + h@R               (PE, during tanh window)
  per group g: th = tanh(HprojT + qT)       (DVE add + ACT tanh, 4 chunks)
               e  = ws-quadrant mms         (PE)
               ex = exp(e) (PSUM->SBUF est) (ACT, no max-sub; e is bounded)
               alphaT[t,b] built from est rows by 4 row-spread SBUF DMAs
                 (est row 32j == alphaT rows 16j..16j+16; gpsimd+sync queues)
               ablk block-diag scatter from alphaT (2 DMA, sync)
               denominator = alphaT^T @ ones (1 matmul) -> reciprocal (DVE)
               ctx = ablk@bHc mms; scale by 1/sum in PSUM->SBUF copy
  xTc = PE-transpose(ctx)
  z  += xTc @ Kc                            (PE)
  gates: sigmoid/tanh (ACT) + c/h (DVE)
Layouts:
  attention world: [128 part = h_lo, 4 h_hi, 64 t, 32 b]
  context world:   [128 part = (b%2)*64 + t, 16 kt=b//2, 512 c]
  LSTM world:      [64 part = b, free]
"""

import numpy as np
import ml_dtypes
from contextlib import ExitStack

B, T, C, H, NCC, S = 512, 64, 512, 512, 96, 26
NCORES = 8
BS = B // NCORES          # 64 batch rows per core
NG = 2                    # groups per core
GB = BS // NG             # 32 rows per group
BF = ml_dtypes.bfloat16

_CACHE = {}


def build_bass():
    import concourse.bass as bass
    import concourse.bacc as bacc
    import concourse.tile as tile
    import concourse.mybir as mybir

    f32 = mybir.dt.float32
    bf16 = mybir.dt.bfloat16
    AF = mybir.ActivationFunctionType
    AX = mybir.AxisListType

    nc = bacc.Bacc("TRN2", target_bir_lowering=False)

    # ---- DRAM I/O ----
    bHT_d = nc.dram_tensor("bHT", [NG, C, T, GB], bf16, kind="ExternalInput")
    bHc_d = nc.dram_tensor("bHc", [NG, GB // 2, 128, C], bf16, kind="ExternalInput")
    wi_d = nc.dram_tensor("wi", [C, H], bf16, kind="ExternalInput")
    wh_d = nc.dram_tensor("wh", [H, H], bf16, kind="ExternalInput")
    bh_d = nc.dram_tensor("bh", [128, 4], f32, kind="ExternalInput")
    ws_d = nc.dram_tensor("ws", [128, 4, 32], bf16, kind="ExternalInput")
    kc_d = nc.dram_tensor("kc", [C, 4 * H], bf16, kind="ExternalInput")
    rr_d = nc.dram_tensor("rr", [H, 4 * H], bf16, kind="ExternalInput")
    ko_d = nc.dram_tensor("ko", [NCC, 4 * H], bf16, kind="ExternalInput")
    oh_d = nc.dram_tensor("oh", [NCC, S, BS], bf16, kind="ExternalInput")
    wg_d = nc.dram_tensor("wg", [H, NCC], bf16, kind="ExternalInput")
    bg_d = nc.dram_tensor("bg", [BS, NCC], f32, kind="ExternalInput")
    id_d = nc.dram_tensor("ident", [128, 128], bf16, kind="ExternalInput")
    out_d = nc.dram_tensor("out", [BS, S, NCC], f32, kind="ExternalOutput")

    NCH = T * GB // 512  # 4 (t,b)-chunks of 512 per group

    with tile.TileContext(nc) as tc, ExitStack() as ctx:
        big = ctx.enter_context(tc.tile_pool(name="big", bufs=1))
        wpool = ctx.enter_context(tc.tile_pool(name="wpool", bufs=1))
        small = ctx.enter_context(tc.tile_pool(name="small", bufs=2))
        tiny = ctx.enter_context(tc.tile_pool(name="tiny", bufs=4))
        gates = ctx.enter_context(tc.tile_pool(name="gates", bufs=4))
        state = ctx.enter_context(tc.tile_pool(name="state", bufs=2))
        # PSUM pools (8 banks total):
        #   pz:  FI + GO gate accumulators  [128,512] x2     = 2 banks
        #   pep: e quadrant accumulator     [128,512] bufs=2 = 2 banks
        #   pcp: ctx accumulator            [32,512]  bufs=1 = 1 bank
        #   ptp: bf16 PE-transpose outs     [128,256] bufs=1 = 1 bank
        #   psm: qT/probs/sums f32 mm outs  [128,256] bufs=2 = 2 banks
        pz = ctx.enter_context(tc.tile_pool(name="pz", bufs=1, space="PSUM"))
        pep = ctx.enter_context(tc.tile_pool(name="pep", bufs=2, space="PSUM"))
        pcp = ctx.enter_context(tc.tile_pool(name="pcp", bufs=1, space="PSUM"))
        ptp = ctx.enter_context(tc.tile_pool(name="ptp", bufs=1, space="PSUM"))
        psm = ctx.enter_context(tc.tile_pool(name="psm", bufs=2, space="PSUM"))

        dma = nc.sync
        import concourse.bass as _b

        # ---- load weights / big tensors ----
        bHc = [big.tile([128, GB // 2, C], bf16, tag=f"bHc{g}", name=f"bHc{g}")
               for g in range(NG)]
        for g in range(NG):
            dma.dma_start(out=bHc[g], in_=bHc_d[g].rearrange("k p c -> p k c"))
        # batch_H^T (prolog only; shares slots with tanh buffers)
        bHT = [big.tile([128, 4, T * GB], bf16, tag=f"th{g}", name=f"bHT{g}")
               for g in range(NG)]
        for g in range(NG):
            dma.dma_start(
                out=bHT[g],
                in_=bHT_d[g].rearrange("(ch cl) t b -> cl ch (t b)", cl=128))

        wi = wpool.tile([128, 4, H], bf16, tag="wi")
        dma.dma_start(out=wi, in_=wi_d[:].rearrange("(ch cl) h -> cl ch h", cl=128))
        wh = wpool.tile([128, 4, H], bf16, tag="wh")
        dma.dma_start(out=wh, in_=wh_d[:].rearrange("(hh hl) h -> hl hh h", hl=128))
        bh = wpool.tile([128, 4], f32, tag="bh")
        dma.dma_start(out=bh, in_=bh_d[:])
        ws = wpool.tile([128, 4, 32], bf16, tag="ws")
        dma.dma_start(out=ws, in_=ws_d[:])
        kc = wpool.tile([128, 4, 4 * H], bf16, tag="kc")
        dma.dma_start(out=kc, in_=kc_d[:].rearrange("(kh kl) n -> kl kh n", kl=128))
        rr = wpool.tile([128, 4, 4 * H], bf16, tag="rr")
        dma.dma_start(out=rr, in_=rr_d[:].rearrange("(kh kl) n -> kl kh n", kl=128))
        ko = wpool.tile([NCC, 4 * H], bf16, tag="ko")
        dma.dma_start(out=ko, in_=ko_d[:])
        oh = wpool.tile([NCC, S, BS], bf16, tag="oh")
        dma.dma_start(out=oh, in_=oh_d[:])
        wg = wpool.tile([128, 4, NCC], bf16, tag="wg")
        dma.dma_start(out=wg, in_=wg_d[:].rearrange("(hh hl) n -> hl hh n", hl=128))
        bg = wpool.tile([BS, NCC], f32, tag="bg")
        dma.dma_start(out=bg, in_=bg_d[:])
        ident = wpool.tile([128, 128], bf16, tag="ident")
        dma.dma_start(out=ident, in_=id_d[:])
        ones = wpool.tile([T, 1], bf16, tag="ones")
        nc.vector.memset(ones, 1.0)
        pr_all = wpool.tile([BS, S, NCC], f32, tag="pr_all")

        # block-diag alpha holders (zeroed once)
        ablk = [wpool.tile([128, GB // 2, GB], bf16, tag=f"ablk{g}", name=f"ablk{g}")
                for g in range(NG)]
        for g in range(NG):
            nc.vector.memset(ablk[g], 0.0)

        # initial state
        hT = [state.tile([128, 4, BS], bf16, tag="hT", name="hT0")]
        nc.vector.memset(hT[0], 0.0)
        c_st = [state.tile([BS, H], f32, tag="c", name="c0")]
        nc.vector.memset(c_st[0], 0.0)
        hbf = [None]

        # ---- prolog: HprojT[g] = (batch_H @ Wi)^T + bh ----
        hprojT = [big.tile([128, 4, T * GB], bf16, tag=f"hp{g}", name=f"hp{g}")
                  for g in range(NG)]
        for g in range(NG):
            for m in range(4):
                for n in range(NCH):
                    ps = pz.tile([128, 512], f32, tag="FI" if g == 0 else "GO")
                    for k in range(4):
                        nc.tensor.matmul(
                            ps,
                            wi[:, k, m * 128:(m + 1) * 128],
                            bHT[g][:, k, n * 512:(n + 1) * 512],
                            start=(k == 0), stop=(k == 3),
                        )
                    nc.scalar.activation(
                        out=hprojT[g][:, m, n * 512:(n + 1) * 512], in_=ps,
                        func=AF.Identity, bias=bh[:, m:m + 1], scale=1.0,
                    )

        def bcast_t(ap2):
            # [128, GB(b)] -> [128, T(t, stride0), GB(b)]
            return _b.AP(tensor=ap2.tensor, offset=ap2.offset,
                         ap=[ap2.ap[0], [0, T], ap2.ap[1]])

        gate_sl = {"f": 1, "i": 0, "g": 2, "o": 3}
        # gate -> (psum tag, row offset): f/i share FI bank, g/o share GO bank
        gate_loc = {"f": ("FI", 0), "i": ("FI", 64), "g": ("GO", 0), "o": ("GO", 64)}

        def emit_hT_transpose(s):
            # h_bf [64, 512] -> hT [128, 4, 64] via 4 PE transposes
            phT = ptp.tile([128, 256], bf16, tag="tp", name=f"phT_{s}")
            for m in range(4):
                nc.tensor.transpose(phT[:, m * 64:(m + 1) * 64],
                                    hbf[0][:, m * 128:(m + 1) * 128],
                                    ident[0:BS, 0:BS])
            hT[0] = state.tile([128, 4, BS], bf16, tag="hT", name=f"hT_{s}")
            nc.vector.tensor_copy(hT[0], phT)

        def emit_qT(s):
            # qT[h',b] = sum_h Wh[h,h'] hT[h,b]; m-outer so chunk m is
            # copied out as soon as its k-accumulation finishes.
            pqT = psm.tile([128, 256], f32, tag="pq", name=f"pqT_{s}")
            qT = small.tile([128, 4, BS], bf16, tag="qT", bufs=2, name=f"qT_{s}")
            for m in range(4):
                for k in range(4):
                    nc.tensor.matmul(pqT[:, m * 64:(m + 1) * 64],
                                     wh[:, k, m * 128:(m + 1) * 128],
                                     hT[0][:, k, :],
                                     start=(k == 0), stop=(k == 3))
                nc.vector.tensor_copy(qT[:, m, :], pqT[:, m * 64:(m + 1) * 64])
            return qT

        def emit_probs(sm1):
            # probs(sm1) = h(sm1) @ Wgen + bg, from hT
            pp = psm.tile([128, 256], f32, tag="pq", name=f"pp_{sm1}")
            for k in range(4):
                nc.tensor.matmul(pp[0:BS, 0:NCC], hT[0][:, k, :], wg[:, k, :],
                                 start=(k == 0), stop=(k == 3))
            nc.vector.tensor_add(pr_all[:, sm1, :], pp[0:BS, 0:NCC], bg)

        def emit_z_early(s, pzt):
            # onehot@Ko' (start) + h@R during the tanh window
            for gn in "figo":
                tag, ro = gate_loc[gn]
                zsl = slice(gate_sl[gn] * 512, (gate_sl[gn] + 1) * 512)
                nc.tensor.matmul(pzt[tag][ro:ro + 64, :], oh[:, s, :],
                                 ko[:, zsl], start=True, stop=False,
                                 tile_position=(0, ro))
            for k in range(4):
                for gn in "figo":
                    tag, ro = gate_loc[gn]
                    zsl = slice(gate_sl[gn] * 512, (gate_sl[gn] + 1) * 512)
                    nc.tensor.matmul(pzt[tag][ro:ro + 64, :], hT[0][:, k, :],
                                     rr[:, k, zsl], start=False, stop=False,
                                     tile_position=(0, ro))

        def emit_att_tanh(s, g, qT, pe_):
            # DVE add + ACT tanh + e quadrant mms for group g
            gsl_b = slice(g * GB, (g + 1) * GB)
            th = big.tile([128, 4, T * GB], bf16, tag=f"th{g}", name=f"th{g}_{s}")
            for k in range(4):
                nc.vector.tensor_add(
                    th[:, k, :].rearrange("p (t b) -> p t b", t=T),
                    hprojT[g][:, k, :].rearrange("p (t b) -> p t b", t=T),
                    bcast_t(qT[:, k, gsl_b]))
                nc.scalar.activation(out=th[:, k, :], in_=th[:, k, :], func=AF.Tanh)
                for j in range(NCH):
                    bp = 32 * j
                    nc.tensor.matmul(pe_[bp:bp + 32, :], ws[:, k, :],
                                     th[:, k, j * 512:(j + 1) * 512],
                                     start=(k == 0), stop=(k == 3),
                                     tile_position=(0, bp))

        def emit_exp_scatter(s, g, pe_):
            # exp on the PSUM layout: est[32j, tl*32+b] = ex(t=16j+tl, b).
            # ablk (block-diag) is written DIRECTLY from est (2 DMAs, one
            # per b-parity); alphaT (only feeds the denominator matmul)
            # via one merged DMA on the gpsimd queue.
            est = small.tile([128, 512], bf16, tag=f"est{g}", bufs=1,
                             name=f"est{g}_{s}")
            nc.scalar.activation(out=est, in_=pe_, func=AF.Exp)
            ea = est[:]
            pp = ea.ap[0][0]
            alphaT = small.tile([T, GB], bf16, tag=f"alphaT{g}", bufs=2,
                                name=f"alphaT{g}_{s}")
            at = alphaT[:]
            for j in range(4):
                esl = est[32 * j:32 * j + 1, :]
                srcj = _b.AP(tensor=esl.tensor, offset=esl.offset,
                             ap=[[esl.ap[0][0], 1], [GB, T // 4], [1, GB]])
                eng = nc.gpsimd if j % 2 == 0 else dma
                eng.dma_start(out=alphaT[16 * j:16 * (j + 1), :], in_=srcj)
            # ablk block-diag scatter (2 DMAs, sync queue)
            ab = ablk[g][:]
            for par in (0, 1):
                srcp = _b.AP(tensor=at.tensor, offset=at.offset + par * at.ap[1][0],
                             ap=[[at.ap[0][0], T], [2 * at.ap[1][0], GB // 2]])
                dst = _b.AP(tensor=ab.tensor,
                            offset=ab.offset + par * (64 * ab.ap[0][0] + ab.ap[2][0]),
                            ap=[[ab.ap[0][0], T], [ab.ap[1][0] + 2 * ab.ap[2][0], GB // 2]])
                dma.dma_start(out=dst, in_=srcp)
            return alphaT

        def emit_post(s, g, ctx_sb, alphaT, psums, pctx, rcp_t):
            # denominator: sums[b] = alphaT^T @ ones (one matmul, N=1);
            # group g occupies psum rows g*GB..(g+1)*GB via array-col
            # quadrant (0, g*GB) so the two groups' ctx streams overlap
            ro = g * GB
            nc.tensor.matmul(psums[ro:ro + GB, 0:1], alphaT, ones,
                             start=True, stop=True, tile_position=(0, ro))
            nc.vector.reciprocal(rcp_t[ro:ro + GB, :], psums[ro:ro + GB, 0:1])
            for kt in range(GB // 2):
                nc.tensor.matmul(pctx[ro:ro + GB, :], ablk[g][:, kt, :],
                                 bHc[g][:, kt, :],
                                 start=(kt == 0), stop=(kt == GB // 2 - 1),
                                 tile_position=(0, ro))
            nc.vector.tensor_scalar_mul(ctx_sb[ro:ro + GB, :],
                                        pctx[ro:ro + GB, :],
                                        rcp_t[ro:ro + GB, :])

        def emit_ctxT(s, g, ctx_sb, pxT, xTc):
            # ctx rows of group g -> xTc[:, k, g*32:(g+1)*32]
            ro = g * GB
            for k in range(4):
                nc.tensor.transpose(pxT[g][:, k * GB:(k + 1) * GB],
                                    ctx_sb[ro:ro + GB, k * 128:(k + 1) * 128],
                                    ident[ro:ro + GB, ro:ro + GB])
            src = pxT[g][:, 0:128].rearrange("p (k b) -> p k b", k=4)
            nc.vector.tensor_copy(xTc[:, :, g * GB:(g + 1) * GB], src)

        def emit_z_late(s, pzt, xTc):
            for pair in ("fi", "go"):
                for k in range(4):
                    for gn in pair:
                        tag, ro = gate_loc[gn]
                        zsl = slice(gate_sl[gn] * 512, (gate_sl[gn] + 1) * 512)
                        nc.tensor.matmul(pzt[tag][ro:ro + 64, :], xTc[:, k, :],
                                         kc[:, k, zsl], start=False,
                                         stop=(k == 3), tile_position=(0, ro))

        def emit_gates(s, pzt):
            sig = {}
            t1 = t2 = None
            for gn in ("f", "i", "g", "o"):
                tag, ro = gate_loc[gn]
                g_sb = gates.tile([BS, H], f32, tag="gate", bufs=4)
                if gn == "g":
                    nc.scalar.activation(out=g_sb, in_=pzt[tag][ro:ro + 64, :],
                                         func=AF.Tanh)
                else:
                    # sigmoid via tanh: keeps ACT on the exp_and_others table
                    nc.scalar.activation(out=g_sb, in_=pzt[tag][ro:ro + 64, :],
                                         func=AF.Tanh, scale=0.5)
                    nc.vector.tensor_scalar(out=g_sb, in0=g_sb,
                                            scalar1=0.5, scalar2=0.5,
                                            op0=mybir.AluOpType.mult,
                                            op1=mybir.AluOpType.add)
                sig[gn] = g_sb
                if gn == "f":
                    t1 = gates.tile([BS, H], f32, tag="tmp", bufs=2)
                    nc.vector.tensor_mul(t1, sig["f"], c_st[0])
                elif gn == "g":
                    t2 = gates.tile([BS, H], f32, tag="tmp", bufs=2)
                    nc.vector.tensor_mul(t2, sig["i"], sig["g"])
                    c_st[0] = state.tile([BS, H], f32, tag="c", name=f"c_{s}")
                    nc.vector.tensor_add(c_st[0], t1, t2)
            tc_sb = gates.tile([BS, H], f32, tag="tmp", bufs=2)
            nc.scalar.activation(out=tc_sb, in_=c_st[0], func=AF.Tanh)
            hbf[0] = small.tile([BS, H], bf16, tag="h_bf", bufs=2,
                                name=f"h_bf_{s}")
            nc.vector.tensor_mul(hbf[0], sig["o"], tc_sb)

        for s in range(S):
            if s > 0:
                emit_hT_transpose(s)
            qT = emit_qT(s)
            if s > 0:
                emit_probs(s - 1)
            pzt = {"FI": pz.tile([128, 512], f32, tag="FI", name=f"pzFI_{s}"),
                   "GO": pz.tile([128, 512], f32, tag="GO", name=f"pzGO_{s}")}
            emit_z_early(s, pzt)
            pe_ = [None] * NG
            ctx_sb = small.tile([BS, C], bf16, tag="ctx_sb", bufs=2,
                                name=f"ctx_{s}")
            psums = psm.tile([128, 256], f32, tag="pq", name=f"psm_{s}")
            pctx = pcp.tile([128, C], f32, tag="ctx", name=f"pctx_{s}")
            rcp_t = tiny.tile([BS, 1], f32, tag="rcp")
            pxT = [None] * NG
            xTc = small.tile([128, 4, BS], bf16, tag="xTc", bufs=2,
                             name=f"xTc_{s}")
            aTs = [None] * NG
            for g in range(NG):
                pe_[g] = pep.tile([128, 512], f32, tag="pe", name=f"pe{g}_{s}")
                emit_att_tanh(s, g, qT, pe_[g])
                aTs[g] = emit_exp_scatter(s, g, pe_[g])
            for g in range(NG):
                emit_post(s, g, ctx_sb, aTs[g], psums, pctx, rcp_t)
                pxT[g] = ptp.tile([128, 256], bf16, tag="tp", name=f"pxT{g}_{s}")
                emit_ctxT(s, g, ctx_sb, pxT, xTc)
            emit_z_late(s, pzt, xTc)
            emit_gates(s, pzt)
        emit_hT_transpose(S)
        emit_probs(S - 1)
        dma.dma_start(out=out_d[:], in_=pr_all)

    nc.finalize()
    return nc


def _prep_core(inputs, i):
    bsl = slice(i * BS, (i + 1) * BS)
    bh_i = np.asarray(inputs["batch_H"][bsl], np.float32)          # [64, 64, 512]
    text_i = np.asarray(inputs["text"][bsl])                       # [64, 26]
    bh_g = bh_i.reshape(NG, GB, T, C)
    m = {}
    m["bHT"] = np.ascontiguousarray(bh_g.transpose(0, 3, 2, 1)).astype(BF)
    m["bHc"] = np.ascontiguousarray(bh_g.reshape(NG, GB // 2, 128, C)).astype(BF)
    m["wi"] = np.asarray(inputs["Wi"], np.float32).astype(BF)
    m["wh"] = np.asarray(inputs["Wh"], np.float32).astype(BF)
    m["bh"] = np.ascontiguousarray(
        np.asarray(inputs["bh"], np.float32).reshape(4, 128).T)
    wsr = np.ascontiguousarray(
        np.asarray(inputs["Ws"], np.float32)[:, 0].reshape(4, 128).T).astype(BF)
    m["ws"] = np.repeat(wsr[:, :, None], 32, axis=2)
    lk = np.asarray(inputs["lstm_kernel"], np.float32)
    lb = np.asarray(inputs["lstm_bias"], np.float32)
    m["kc"] = lk[:C].astype(BF)
    m["ko"] = (lk[C:] + lb[None, :]).astype(BF)
    m["rr"] = np.asarray(inputs["lstm_rec"], np.float32).astype(BF)
    m["oh"] = (np.arange(NCC)[:, None, None] == text_i.T[None, :, :]).astype(BF)
    m["wg"] = np.asarray(inputs["Wgen"], np.float32).astype(BF)
    m["bg"] = np.tile(np.asarray(inputs["bgen"], np.float32)[None, :], (BS, 1))
    m["ident"] = np.eye(128, dtype=np.float32).astype(BF)
    return m


def kernel(_trace=False, **inputs):
    from concourse import bass_utils
    if "nc" not in _CACHE:
        _CACHE["nc"] = build_bass()
    nc = _CACHE["nc"]
    in_maps = [_prep_core(inputs, i) for i in range(NCORES)]
    res = bass_utils.run_bass_kernel_spmd(nc, in_maps, list(range(NCORES)),
                                          trace=_trace)
    _CACHE["last_result"] = res
    out = np.concatenate([r["out"] for r in res.results], axis=0)
    return out.astype(np.float32)


# revision 34
# speedup vs baseline: 1.0020x; 1.0001x over previous
"""Bass/Trainium2 kernel for attention-LSTM decoder (nn_Attention_49289044688898).

Data-parallel over batch: 512 rows -> 8 NeuronCores x 64 rows. Weights replicated.
Within a core, 64 rows = 2 groups of 32 for the attention; LSTM/q/probs joint.

v2 schedule (vs v1): no DMA transposes (PE transposes + direct-transposed qT
matmul), oh/h@R/probs matmuls hoisted into the tanh window, exp before the
e-scatter DRAM roundtrip with softmax normalization folded into the ctx
PSUM->SBUF copy, AF.Sigmoid for gates.

Per step s (26 steps):
  hT  = transpose(h)                        (PE, 4 transpose-mm)
  qT  = WhT-chunks @ hT                     (PE, 16 mm N=64, k-accum)
  probs(s-1) = hT-mm @ Wgen + bg            (PE + DVE, during tanh window)
  z-partial: onehot@Ko' + h@R               (PE, during tanh window)
  per group g: th = tanh(HprojT + qT)       (DVE add + ACT tanh, 4 chunks)
               e  = ws-quadrant mms         (PE)
               ex = exp(e) (PSUM->SBUF est) (ACT, no max-sub; e is bounded)
               alphaT[t,b] built from est rows by 4 row-spread SBUF DMAs
                 (est row 32j == alphaT rows 16j..16j+16; gpsimd+sync queues)
               ablk block-diag scatter from alphaT (2 DMA, sync)
               denominator = alphaT^T @ ones (1 matmul) -> reciprocal (DVE)
               ctx = ablk@bHc mms; scale by 1/sum in PSUM->SBUF copy
  xTc = PE-transpose(ctx)
  z  += xTc @ Kc                            (PE)
  gates: sigmoid/tanh (ACT) + c/h (DVE)
Layouts:
  attention world: [128 part = h_lo, 4 h_hi, 64 t, 32 b]
  context world:   [128 part = (b%2)*64 + t, 16 kt=b//2, 512 c]
  LSTM world:      [64 part = b, free]
"""

import numpy as np
import ml_dtypes
from contextlib import ExitStack

B, T, C, H, NCC, S = 512, 64, 512, 512, 96, 26
NCORES = 8
BS = B // NCORES          # 64 batch rows per core
NG = 2                    # groups per core
GB = BS // NG             # 32 rows per group
BF = ml_dtypes.bfloat16

_CACHE = {}


def build_bass():
    import concourse.bass as bass
    import concourse.bacc as bacc
    import concourse.tile as tile
    import concourse.mybir as mybir

    f32 = mybir.dt.float32
    bf16 = mybir.dt.bfloat16
    AF = mybir.ActivationFunctionType
    AX = mybir.AxisListType

    nc = bacc.Bacc("TRN2", target_bir_lowering=False)

    # ---- DRAM I/O ----
    bHT_d = nc.dram_tensor("bHT", [NG, C, T, GB], bf16, kind="ExternalInput")
    bHc_d = nc.dram_tensor("bHc", [NG, GB // 2, 128, C], bf16, kind="ExternalInput")
    wi_d = nc.dram_tensor("wi", [C, H], bf16, kind="ExternalInput")
    wh_d = nc.dram_tensor("wh", [H, H], bf16, kind="ExternalInput")
    bh_d = nc.dram_tensor("bh", [128, 4], f32, kind="ExternalInput")
    ws_d = nc.dram_tensor("ws", [128, 4, 32], bf16, kind="ExternalInput")
    kc_d = nc.dram_tensor("kc", [C, 4 * H], bf16, kind="ExternalInput")
    rr_d = nc.dram_tensor("rr", [H, 4 * H], bf16, kind="ExternalInput")
    ko_d = nc.dram_tensor("ko", [NCC, 4 * H], bf16, kind="ExternalInput")
    oh_d = nc.dram_tensor("oh", [NCC, S, BS], bf16, kind="ExternalInput")
    wg_d = nc.dram_tensor("wg", [H, NCC], bf16, kind="ExternalInput")
    bg_d = nc.dram_tensor("bg", [BS, NCC], f32, kind="ExternalInput")
    id_d = nc.dram_tensor("ident", [128, 128], bf16, kind="ExternalInput")
    out_d = nc.dram_tensor("out", [BS, S, NCC], f32, kind="ExternalOutput")

    NCH = T * GB // 512  # 4 (t,b)-chunks of 512 per group

    with tile.TileContext(nc) as tc, ExitStack() as ctx:
        big = ctx.enter_context(tc.tile_pool(name="big", bufs=1))
        wpool = ctx.enter_context(tc.tile_pool(name="wpool", bufs=1))
        small = ctx.enter_context(tc.tile_pool(name="small", bufs=2))
        tiny = ctx.enter_context(tc.tile_pool(name="tiny", bufs=4))
        gates = ctx.enter_context(tc.tile_pool(name="gates", bufs=4))
        state = ctx.enter_context(tc.tile_pool(name="state", bufs=2))
        # PSUM pools (8 banks total):
        #   pz:  FI + GO gate accumulators  [128,512] x2     = 2 banks
        #   pep: e quadrant accumulator     [128,512] bufs=2 = 2 banks
        #   pcp: ctx accumulator            [32,512]  bufs=1 = 1 bank
        #   ptp: bf16 PE-transpose outs     [128,256] bufs=1 = 1 bank
        #   psm: qT/probs/sums f32 mm outs  [128,256] bufs=2 = 2 banks
        pz = ctx.enter_context(tc.tile_pool(name="pz", bufs=1, space="PSUM"))
        pep = ctx.enter_context(tc.tile_pool(name="pep", bufs=2, space="PSUM"))
        pcp = ctx.enter_context(tc.tile_pool(name="pcp", bufs=1, space="PSUM"))
        ptp = ctx.enter_context(tc.tile_pool(name="ptp", bufs=1, space="PSUM"))
        psm = ctx.enter_context(tc.tile_pool(name="psm", bufs=2, space="PSUM"))

        dma = nc.sync
        import concourse.bass as _b

        # ---- load weights / big tensors ----
        bHc = [big.tile([128, GB // 2, C], bf16, tag=f"bHc{g}", name=f"bHc{g}")
               for g in range(NG)]
        for g in range(NG):
            dma.dma_start(out=bHc[g], in_=bHc_d[g].rearrange("k p c -> p k c"))
        # batch_H^T (prolog only; shares slots with tanh buffers)
        bHT = [big.tile([128, 4, T * GB], bf16, tag=f"th{g}", name=f"bHT{g}")
               for g in range(NG)]
        for g in range(NG):
            dma.dma_start(
                out=bHT[g],
                in_=bHT_d[g].rearrange("(ch cl) t b -> cl ch (t b)", cl=128))

        wi = wpool.tile([128, 4, H], bf16, tag="wi")
        dma.dma_start(out=wi, in_=wi_d[:].rearrange("(ch cl) h -> cl ch h", cl=128))
        wh = wpool.tile([128, 4, H], bf16, tag="wh")
        dma.dma_start(out=wh, in_=wh_d[:].rearrange("(hh hl) h -> hl hh h", hl=128))
        bh = wpool.tile([128, 4], f32, tag="bh")
        dma.dma_start(out=bh, in_=bh_d[:])
        ws = wpool.tile([128, 4, 32], bf16, tag="ws")
        dma.dma_start(out=ws, in_=ws_d[:])
        kc = wpool.tile([128, 4, 4 * H], bf16, tag="kc")
        dma.dma_start(out=kc, in_=kc_d[:].rearrange("(kh kl) n -> kl kh n", kl=128))
        rr = wpool.tile([128, 4, 4 * H], bf16, tag="rr")
        dma.dma_start(out=rr, in_=rr_d[:].rearrange("(kh kl) n -> kl kh n", kl=128))
        ko = wpool.tile([NCC, 4 * H], bf16, tag="ko")
        dma.dma_start(out=ko, in_=ko_d[:])
        oh = wpool.tile([NCC, S, BS], bf16, tag="oh")
        dma.dma_start(out=oh, in_=oh_d[:])
        wg = wpool.tile([128, 4, NCC], bf16, tag="wg")
        dma.dma_start(out=wg, in_=wg_d[:].rearrange("(hh hl) n -> hl hh n", hl=128))
        bg = wpool.tile([BS, NCC], f32, tag="bg")
        dma.dma_start(out=bg, in_=bg_d[:])
        ident = wpool.tile([128, 128], bf16, tag="ident")
        dma.dma_start(out=ident, in_=id_d[:])
        ones = wpool.tile([T, 1], bf16, tag="ones")
        nc.vector.memset(ones, 1.0)
        pr_all = wpool.tile([BS, S, NCC], f32, tag="pr_all")

        # block-diag alpha holders (zeroed once)
        ablk = [wpool.tile([128, GB // 2, GB], bf16, tag=f"ablk{g}", name=f"ablk{g}")
                for g in range(NG)]
        for g in range(NG):
            nc.vector.memset(ablk[g], 0.0)

        # initial state
        hT = [state.tile([128, 4, BS], bf16, tag="hT", name="hT0")]
        nc.vector.memset(hT[0], 0.0)
        c_st = [state.tile([BS, H], f32, tag="c", name="c0")]
        nc.vector.memset(c_st[0], 0.0)
        hbf = [None]

        # ---- prolog: HprojT[g] = (batch_H @ Wi)^T + bh ----
        hprojT = [big.tile([128, 4, T * GB], bf16, tag=f"hp{g}", name=f"hp{g}")
                  for g in range(NG)]
        for g in range(NG):
            for m in range(4):
                for n in range(NCH):
                    ps = pz.tile([128, 512], f32, tag="FI" if g == 0 else "GO")
                    for k in range(4):
                        nc.tensor.matmul(
                            ps,
                            wi[:, k, m * 128:(m + 1) * 128],
                            bHT[g][:, k, n * 512:(n + 1) * 512],
                            start=(k == 0), stop=(k == 3),
                        )
                    if (m + n) % 2 == 0:
                        nc.scalar.activation(
                            out=hprojT[g][:, m, n * 512:(n + 1) * 512], in_=ps,
                            func=AF.Identity, bias=bh[:, m:m + 1], scale=1.0,
                        )
                    else:
                        nc.vector.tensor_scalar_add(
                            hprojT[g][:, m, n * 512:(n + 1) * 512], ps,
                            bh[:, m:m + 1])

        def bcast_t(ap2):
            # [128, GB(b)] -> [128, T(t, stride0), GB(b)]
            return _b.AP(tensor=ap2.tensor, offset=ap2.offset,
                         ap=[ap2.ap[0], [0, T], ap2.ap[1]])

        gate_sl = {"f": 1, "i": 0, "g": 2, "o": 3}
        # gate -> (psum tag, row offset): f/i share FI bank, g/o share GO bank
        gate_loc = {"f": ("FI", 0), "i": ("FI", 64), "g": ("GO", 0), "o": ("GO", 64)}

        def emit_hT_transpose(s):
            # h_bf [64, 512] -> hT [128, 4, 64] via 4 PE transposes
            phT = ptp.tile([128, 256], bf16, tag="tp", name=f"phT_{s}")
            for m in range(4):
                nc.tensor.transpose(phT[:, m * 64:(m + 1) * 64],
                                    hbf[0][:, m * 128:(m + 1) * 128],
                                    ident[0:BS, 0:BS])
            hT[0] = state.tile([128, 4, BS], bf16, tag="hT", name=f"hT_{s}")
            nc.vector.tensor_copy(hT[0], phT)

        def emit_qT(s):
            # qT[h',b] = sum_h Wh[h,h'] hT[h,b]; m-outer so chunk m is
            # copied out as soon as its k-accumulation finishes.
            pqT = psm.tile([128, 256], f32, tag="pq", name=f"pqT_{s}")
            qT = small.tile([128, 4, BS], bf16, tag="qT", bufs=2, name=f"qT_{s}")
            for m in range(4):
                for k in range(4):
                    nc.tensor.matmul(pqT[:, m * 64:(m + 1) * 64],
                                     wh[:, k, m * 128:(m + 1) * 128],
                                     hT[0][:, k, :],
                                     start=(k == 0), stop=(k == 3))
                nc.vector.tensor_copy(qT[:, m, :], pqT[:, m * 64:(m + 1) * 64])
            return qT

        def emit_probs(sm1):
            # probs(sm1) = h(sm1) @ Wgen + bg, from hT
            pp = psm.tile([128, 256], f32, tag="pq", name=f"pp_{sm1}")
            for k in range(4):
                nc.tensor.matmul(pp[0:BS, 0:NCC], hT[0][:, k, :], wg[:, k, :],
                                 start=(k == 0), stop=(k == 3))
            nc.vector.tensor_add(pr_all[:, sm1, :], pp[0:BS, 0:NCC], bg)

        def emit_z_early(s, pzt):
            # onehot@Ko' (start) + h@R during the tanh window
            for gn in "figo":
                tag, ro = gate_loc[gn]
                zsl = slice(gate_sl[gn] * 512, (gate_sl[gn] + 1) * 512)
                nc.tensor.matmul(pzt[tag][ro:ro + 64, :], oh[:, s, :],
                                 ko[:, zsl], start=True, stop=False,
                                 tile_position=(0, ro))
            if s == 0:
                return  # h == 0: R contributes nothing
            for k in range(4):
                for gn in "figo":
                    tag, ro = gate_loc[gn]
                    zsl = slice(gate_sl[gn] * 512, (gate_sl[gn] + 1) * 512)
                    nc.tensor.matmul(pzt[tag][ro:ro + 64, :], hT[0][:, k, :],
                                     rr[:, k, zsl], start=False, stop=False,
                                     tile_position=(0, ro))

        def emit_att_tanh(s, g, qT, pe_):
            # DVE add + ACT tanh + e quadrant mms for group g
            gsl_b = slice(g * GB, (g + 1) * GB)
            th = big.tile([128, 4, T * GB], bf16, tag=f"th{g}", name=f"th{g}_{s}")
            for k in range(4):
                if qT is None:
                    # step 0: q == 0, tanh straight off hprojT
                    nc.scalar.activation(out=th[:, k, :],
                                         in_=hprojT[g][:, k, :], func=AF.Tanh)
                else:
                    nc.vector.tensor_add(
                        th[:, k, :].rearrange("p (t b) -> p t b", t=T),
                        hprojT[g][:, k, :].rearrange("p (t b) -> p t b", t=T),
                        bcast_t(qT[:, k, gsl_b]))
                    nc.scalar.activation(out=th[:, k, :], in_=th[:, k, :],
                                         func=AF.Tanh)
                for j in range(NCH):
                    bp = 32 * j
                    nc.tensor.matmul(pe_[bp:bp + 32, :], ws[:, k, :],
                                     th[:, k, j * 512:(j + 1) * 512],
                                     start=(k == 0), stop=(k == 3),
                                     tile_position=(0, bp))

        def emit_exp_scatter(s, g, pe_):
            # exp on the PSUM layout: est[32j, tl*32+b] = ex(t=16j+tl, b).
            # ablk (block-diag) is written DIRECTLY from est (2 DMAs, one
            # per b-parity); alphaT (only feeds the denominator matmul)
            # via one merged DMA on the gpsimd queue.
            est = small.tile([128, 512], bf16, tag=f"est{g}", bufs=1,
                             name=f"est{g}_{s}")
            nc.scalar.activation(out=est, in_=pe_, func=AF.Exp)
            ea = est[:]
            pp = ea.ap[0][0]
            alphaT = small.tile([T, GB], bf16, tag=f"alphaT{g}", bufs=2,
                                name=f"alphaT{g}_{s}")
            at = alphaT[:]
            for j in range(4):
                esl = est[32 * j:32 * j + 1, :]
                srcj = _b.AP(tensor=esl.tensor, offset=esl.offset,
                             ap=[[esl.ap[0][0], 1], [GB, T // 4], [1, GB]])
                eng = nc.gpsimd if j % 2 == 0 else dma
                eng.dma_start(out=alphaT[16 * j:16 * (j + 1), :], in_=srcj)
            # ablk block-diag scatter (2 DMAs, sync queue)
            ab = ablk[g][:]
            for par in (0, 1):
                srcp = _b.AP(tensor=at.tensor, offset=at.offset + par * at.ap[1][0],
                             ap=[[at.ap[0][0], T], [2 * at.ap[1][0], GB // 2]])
                dst = _b.AP(tensor=ab.tensor,
                            offset=ab.offset + par * (64 * ab.ap[0][0] + ab.ap[2][0]),
                            ap=[[ab.ap[0][0], T], [ab.ap[1][0] + 2 * ab.ap[2][0], GB // 2]])
                dma.dma_start(out=dst, in_=srcp)
            return alphaT

        def emit_post(s, g, ctx_sb, alphaT, psums, pctx, rcp_t):
            # denominator: sums[b] = alphaT^T @ ones (one matmul, N=1);
            # group g occupies psum rows g*GB..(g+1)*GB via array-col
            # quadrant (0, g*GB) so the two groups' ctx streams overlap
            ro = g * GB
            nc.tensor.matmul(psums[ro:ro + GB, 0:1], alphaT, ones,
                             start=True, stop=True, tile_position=(0, ro))
            nc.vector.reciprocal(rcp_t[ro:ro + GB, :], psums[ro:ro + GB, 0:1])
            for kt in range(GB // 2):
                nc.tensor.matmul(pctx[ro:ro + GB, :], ablk[g][:, kt, :],
                                 bHc[g][:, kt, :],
                                 start=(kt == 0), stop=(kt == GB // 2 - 1),
                                 tile_position=(0, ro))
            nc.vector.tensor_scalar_mul(ctx_sb[ro:ro + GB, :],
                                        pctx[ro:ro + GB, :],
                                        rcp_t[ro:ro + GB, :])

        def emit_ctxT(s, g, ctx_sb, pxT, xTc):
            # ctx rows of group g -> xTc[:, k, g*32:(g+1)*32]
            ro = g * GB
            for k in range(4):
                nc.tensor.transpose(pxT[g][:, k * GB:(k + 1) * GB],
                                    ctx_sb[ro:ro + GB, k * 128:(k + 1) * 128],
                                    ident[ro:ro + GB, ro:ro + GB])
            src = pxT[g][:, 0:128].rearrange("p (k b) -> p k b", k=4)
            nc.vector.tensor_copy(xTc[:, :, g * GB:(g + 1) * GB], src)

        def emit_z_late(s, pzt, xTc):
            for pair in ("fi", "go"):
                for k in range(4):
                    for gn in pair:
                        tag, ro = gate_loc[gn]
                        zsl = slice(gate_sl[gn] * 512, (gate_sl[gn] + 1) * 512)
                        nc.tensor.matmul(pzt[tag][ro:ro + 64, :], xTc[:, k, :],
                                         kc[:, k, zsl], start=False,
                                         stop=(k == 3), tile_position=(0, ro))

        def emit_gates(s, pzt):
            sig = {}
            t1 = t2 = None
            for gn in ("f", "i", "g", "o"):
                tag, ro = gate_loc[gn]
                g_sb = gates.tile([BS, H], f32, tag="gate", bufs=4)
                if gn == "g":
                    nc.scalar.activation(out=g_sb, in_=pzt[tag][ro:ro + 64, :],
                                         func=AF.Tanh)
                else:
                    # sigmoid via tanh: keeps ACT on the exp_and_others table
                    nc.scalar.activation(out=g_sb, in_=pzt[tag][ro:ro + 64, :],
                                         func=AF.Tanh, scale=0.5)
                    nc.vector.tensor_scalar(out=g_sb, in0=g_sb,
                                            scalar1=0.5, scalar2=0.5,
                                            op0=mybir.AluOpType.mult,
                                            op1=mybir.AluOpType.add)
                sig[gn] = g_sb
                if gn == "f":
                    t1 = gates.tile([BS, H], f32, tag="tmp", bufs=2)
                    nc.vector.tensor_mul(t1, sig["f"], c_st[0])
                elif gn == "g":
                    t2 = gates.tile([BS, H], f32, tag="tmp", bufs=2)
                    nc.vector.tensor_mul(t2, sig["i"], sig["g"])
                    c_st[0] = state.tile([BS, H], f32, tag="c", name=f"c_{s}")
                    nc.vector.tensor_add(c_st[0], t1, t2)
            tc_sb = gates.tile([BS, H], f32, tag="tmp", bufs=2)
            nc.scalar.activation(out=tc_sb, in_=c_st[0], func=AF.Tanh)
            hbf[0] = small.tile([BS, H], bf16, tag="h_bf", bufs=2,
                                name=f"h_bf_{s}")
            nc.vector.tensor_mul(hbf[0], sig["o"], tc_sb)

        for s in range(S):
            if s > 0:
                emit_hT_transpose(s)
                qT = emit_qT(s)
                emit_probs(s - 1)
            else:
                qT = None
            pzt = {"FI": pz.tile([128, 512], f32, tag="FI", name=f"pzFI_{s}"),
                   "GO": pz.tile([128, 512], f32, tag="GO", name=f"pzGO_{s}")}
            emit_z_early(s, pzt)
            pe_ = [None] * NG
            ctx_sb = small.tile([BS, C], bf16, tag="ctx_sb", bufs=2,
                                name=f"ctx_{s}")
            psums = psm.tile([128, 256], f32, tag="pq", name=f"psm_{s}")
            pctx = pcp.tile([128, C], f32, tag="ctx", name=f"pctx_{s}")
            rcp_t = tiny.tile([BS, 1], f32, tag="rcp")
            pxT = [None] * NG
            xTc = small.tile([128, 4, BS], bf16, tag="xTc", bufs=2,
                             name=f"xTc_{s}")
            aTs = [None] * NG
            for g in range(NG):
                pe_[g] = pep.tile([128, 512], f32, tag="pe", name=f"pe{g}_{s}")
                emit_att_tanh(s, g, qT, pe_[g])
                aTs[g] = emit_exp_scatter(s, g, pe_[g])
            for g in range(NG):
                emit_post(s, g, ctx_sb, aTs[g], psums, pctx, rcp_t)
                pxT[g] = ptp.tile([128, 256], bf16, tag="tp", name=f"pxT{g}_{s}")
                emit_ctxT(s, g, ctx_sb, pxT, xTc)
            emit_z_late(s, pzt, xTc)
            emit_gates(s, pzt)
        emit_hT_transpose(S)
        emit_probs(S - 1)
        dma.dma_start(out=out_d[:], in_=pr_all)

    nc.finalize()
    return nc


def _prep_core(inputs, i):
    bsl = slice(i * BS, (i + 1) * BS)
    bh_i = np.asarray(inputs["batch_H"][bsl], np.float32)          # [64, 64, 512]
    text_i = np.asarray(inputs["text"][bsl])                       # [64, 26]
    bh_g = bh_i.reshape(NG, GB, T, C)
    m = {}
    m["bHT"] = np.ascontiguousarray(bh_g.transpose(0, 3, 2, 1)).astype(BF)
    m["bHc"] = np.ascontiguousarray(bh_g.reshape(NG, GB // 2, 128, C)).astype(BF)
    m["wi"] = np.asarray(inputs["Wi"], np.float32).astype(BF)
    m["wh"] = np.asarray(inputs["Wh"], np.float32).astype(BF)
    m["bh"] = np.ascontiguousarray(
        np.asarray(inputs["bh"], np.float32).reshape(4, 128).T)
    wsr = np.ascontiguousarray(
        np.asarray(inputs["Ws"], np.float32)[:, 0].reshape(4, 128).T).astype(BF)
    m["ws"] = np.repeat(wsr[:, :, None], 32, axis=2)
    lk = np.asarray(inputs["lstm_kernel"], np.float32)
    lb = np.asarray(inputs["lstm_bias"], np.float32)
    m["kc"] = lk[:C].astype(BF)
    m["ko"] = (lk[C:] + lb[None, :]).astype(BF)
    m["rr"] = np.asarray(inputs["lstm_rec"], np.float32).astype(BF)
    m["oh"] = (np.arange(NCC)[:, None, None] == text_i.T[None, :, :]).astype(BF)
    m["wg"] = np.asarray(inputs["Wgen"], np.float32).astype(BF)
    m["bg"] = np.tile(np.asarray(inputs["bgen"], np.float32)[None, :], (BS, 1))
    m["ident"] = np.eye(128, dtype=np.float32).astype(BF)
    return m


def kernel(_trace=False, **inputs):
    from concourse import bass_utils
    if "nc" not in _CACHE:
        _CACHE["nc"] = build_bass()
    nc = _CACHE["nc"]
    in_maps = [_prep_core(inputs, i) for i in range(NCORES)]
    res = bass_utils.run_bass_kernel_spmd(nc, in_maps, list(range(NCORES)),
                                          trace=_trace)
    _CACHE["last_result"] = res
    out = np.concatenate([r["out"] for r in res.results], axis=0)
    return out.astype(np.float32)


# revision 36
# speedup vs baseline: 1.0050x; 1.0030x over previous
"""Bass/Trainium2 kernel for attention-LSTM decoder (nn_Attention_49289044688898).

Data-parallel over batch: 512 rows -> 8 NeuronCores x 64 rows. Weights replicated.
Within a core, 64 rows = 2 groups of 32 for the attention; LSTM/q/probs joint.

v2 schedule (vs v1): no DMA transposes (PE transposes + direct-transposed qT
matmul), oh/h@R/probs matmuls hoisted into the tanh window, exp before the
e-scatter DRAM roundtrip with softmax normalization folded into the ctx
PSUM->SBUF copy, AF.Sigmoid for gates.

Per step s (26 steps):
  hT  = transpose(h)                        (PE, 4 transpose-mm)
  qT  = WhT-chunks @ hT                     (PE, 16 mm N=64, k-accum)
  probs(s-1) = hT-mm @ Wgen + bg            (PE + DVE, during tanh window)
  z-partial: onehot@Ko' + h@R               (PE, during tanh window)
  per group g: th = tanh(HprojT + qT)       (DVE add + ACT tanh, 4 chunks)
               e  = ws-quadrant mms         (PE)
               ex = exp(e) (PSUM->SBUF est) (ACT, no max-sub; e is bounded)
               alphaT[t,b] built from est rows by 4 row-spread SBUF DMAs
                 (est row 32j == alphaT rows 16j..16j+16; gpsimd+sync queues)
               ablk block-diag scatter from alphaT (2 DMA, sync)
               denominator = alphaT^T @ ones (1 matmul) -> reciprocal (DVE)
               ctx = ablk@bHc mms; scale by 1/sum in PSUM->SBUF copy
  xTc = PE-transpose(ctx)
  z  += xTc @ Kc                            (PE)
  gates: sigmoid/tanh (ACT) + c/h (DVE)
Layouts:
  attention world: [128 part = h_lo, 4 h_hi, 64 t, 32 b]
  context world:   [128 part = (b%2)*64 + t, 16 kt=b//2, 512 c]
  LSTM world:      [64 part = b, free]
"""

import numpy as np
import ml_dtypes
from contextlib import ExitStack

B, T, C, H, NCC, S = 512, 64, 512, 512, 96, 26
NCORES = 8
BS = B // NCORES          # 64 batch rows per core
NG = 2                    # groups per core
GB = BS // NG             # 32 rows per group
BF = ml_dtypes.bfloat16

_CACHE = {}


def build_bass():
    import concourse.bass as bass
    import concourse.bacc as bacc
    import concourse.tile as tile
    import concourse.mybir as mybir

    f32 = mybir.dt.float32
    bf16 = mybir.dt.bfloat16
    AF = mybir.ActivationFunctionType
    AX = mybir.AxisListType

    nc = bacc.Bacc("TRN2", target_bir_lowering=False)

    # ---- DRAM I/O ----
    bHT_d = nc.dram_tensor("bHT", [NG, C, T, GB], bf16, kind="ExternalInput")
    bHc_d = nc.dram_tensor("bHc", [NG, GB // 2, 128, C], bf16, kind="ExternalInput")
    wi_d = nc.dram_tensor("wi", [C, H], bf16, kind="ExternalInput")
    wh_d = nc.dram_tensor("wh", [H, H], bf16, kind="ExternalInput")
    bh_d = nc.dram_tensor("bh", [128, 4], f32, kind="ExternalInput")
    ws_d = nc.dram_tensor("ws", [128, 4, 32], bf16, kind="ExternalInput")
    kc_d = nc.dram_tensor("kc", [C, 4 * H], bf16, kind="ExternalInput")
    rr_d = nc.dram_tensor("rr", [H, 4 * H], bf16, kind="ExternalInput")
    ko_d = nc.dram_tensor("ko", [NCC, 4 * H], bf16, kind="ExternalInput")
    oh_d = nc.dram_tensor("oh", [NCC, S, BS], bf16, kind="ExternalInput")
    wg_d = nc.dram_tensor("wg", [H, NCC], bf16, kind="ExternalInput")
    bg_d = nc.dram_tensor("bg", [BS, NCC], f32, kind="ExternalInput")
    id_d = nc.dram_tensor("ident", [128, 128], bf16, kind="ExternalInput")
    out_d = nc.dram_tensor("out", [BS, S, NCC], f32, kind="ExternalOutput")

    NCH = T * GB // 512  # 4 (t,b)-chunks of 512 per group

    with tile.TileContext(nc) as tc, ExitStack() as ctx:
        big = ctx.enter_context(tc.tile_pool(name="big", bufs=1))
        wpool = ctx.enter_context(tc.tile_pool(name="wpool", bufs=1))
        small = ctx.enter_context(tc.tile_pool(name="small", bufs=2))
        tiny = ctx.enter_context(tc.tile_pool(name="tiny", bufs=4))
        gates = ctx.enter_context(tc.tile_pool(name="gates", bufs=4))
        state = ctx.enter_context(tc.tile_pool(name="state", bufs=2))
        # PSUM pools (8 banks total):
        #   pz:  FI + GO gate accumulators  [128,512] x2     = 2 banks
        #   pep: e quadrant accumulator     [128,512] bufs=2 = 2 banks
        #   pcp: ctx accumulator            [32,512]  bufs=1 = 1 bank
        #   ptp: bf16 PE-transpose outs     [128,256] bufs=1 = 1 bank
        #   psm: qT/probs/sums f32 mm outs  [128,256] bufs=2 = 2 banks
        pz = ctx.enter_context(tc.tile_pool(name="pz", bufs=1, space="PSUM"))
        pep = ctx.enter_context(tc.tile_pool(name="pep", bufs=2, space="PSUM"))
        pcp = ctx.enter_context(tc.tile_pool(name="pcp", bufs=1, space="PSUM"))
        ptp = ctx.enter_context(tc.tile_pool(name="ptp", bufs=1, space="PSUM"))
        psm = ctx.enter_context(tc.tile_pool(name="psm", bufs=2, space="PSUM"))

        dma = nc.sync
        import concourse.bass as _b

        # ---- load weights / big tensors ----
        bHc = [big.tile([128, GB // 2, C], bf16, tag=f"bHc{g}", name=f"bHc{g}")
               for g in range(NG)]
        for g in range(NG):
            dma.dma_start(out=bHc[g], in_=bHc_d[g].rearrange("k p c -> p k c"))
        # batch_H^T (prolog only; shares slots with tanh buffers)
        bHT = [big.tile([128, 4, T * GB], bf16, tag=f"th{g}", name=f"bHT{g}")
               for g in range(NG)]
        for g in range(NG):
            dma.dma_start(
                out=bHT[g],
                in_=bHT_d[g].rearrange("(ch cl) t b -> cl ch (t b)", cl=128))

        wi = wpool.tile([128, 4, H], bf16, tag="wi")
        dma.dma_start(out=wi, in_=wi_d[:].rearrange("(ch cl) h -> cl ch h", cl=128))
        wh = wpool.tile([128, 4, H], bf16, tag="wh")
        dma.dma_start(out=wh, in_=wh_d[:].rearrange("(hh hl) h -> hl hh h", hl=128))
        bh = wpool.tile([128, 4], f32, tag="bh")
        dma.dma_start(out=bh, in_=bh_d[:])
        ws = wpool.tile([128, 4, 32], bf16, tag="ws")
        dma.dma_start(out=ws, in_=ws_d[:])
        kc = wpool.tile([128, 4, 4 * H], bf16, tag="kc")
        dma.dma_start(out=kc, in_=kc_d[:].rearrange("(kh kl) n -> kl kh n", kl=128))
        rr = wpool.tile([128, 4, 4 * H], bf16, tag="rr")
        dma.dma_start(out=rr, in_=rr_d[:].rearrange("(kh kl) n -> kl kh n", kl=128))
        ko = wpool.tile([NCC, 4 * H], bf16, tag="ko")
        dma.dma_start(out=ko, in_=ko_d[:])
        oh = wpool.tile([NCC, S, BS], bf16, tag="oh")
        dma.dma_start(out=oh, in_=oh_d[:])
        wg = wpool.tile([128, 4, NCC], bf16, tag="wg")
        dma.dma_start(out=wg, in_=wg_d[:].rearrange("(hh hl) n -> hl hh n", hl=128))
        bg = wpool.tile([BS, NCC], f32, tag="bg")
        dma.dma_start(out=bg, in_=bg_d[:])
        ident = wpool.tile([128, 128], bf16, tag="ident")
        dma.dma_start(out=ident, in_=id_d[:])
        ones = wpool.tile([T, 1], bf16, tag="ones")
        nc.vector.memset(ones, 1.0)
        pr_all = wpool.tile([BS, S, NCC], f32, tag="pr_all")

        # block-diag alpha holders (zeroed once)
        ablk = [wpool.tile([128, GB // 2, GB], bf16, tag=f"ablk{g}", name=f"ablk{g}")
                for g in range(NG)]
        for g in range(NG):
            nc.vector.memset(ablk[g], 0.0)

        # initial state
        hT = [state.tile([128, 4, BS], bf16, tag="hT", name="hT0")]
        nc.vector.memset(hT[0], 0.0)
        c_st = [state.tile([BS, H], f32, tag="c", name="c0")]
        nc.vector.memset(c_st[0], 0.0)
        hbf = [None]

        # ---- prolog: HprojT[g] = (batch_H @ Wi)^T + bh ----
        hprojT = [big.tile([128, 4, T * GB], bf16, tag=f"hp{g}", name=f"hp{g}")
                  for g in range(NG)]
        for g in range(NG):
            for m in range(4):
                for n in range(NCH):
                    ps = pz.tile([128, 512], f32, tag="FI" if g == 0 else "GO")
                    for k in range(4):
                        nc.tensor.matmul(
                            ps,
                            wi[:, k, m * 128:(m + 1) * 128],
                            bHT[g][:, k, n * 512:(n + 1) * 512],
                            start=(k == 0), stop=(k == 3),
                        )
                    if (m + n) % 2 == 0:
                        nc.scalar.activation(
                            out=hprojT[g][:, m, n * 512:(n + 1) * 512], in_=ps,
                            func=AF.Identity, bias=bh[:, m:m + 1], scale=1.0,
                        )
                    else:
                        nc.vector.tensor_scalar_add(
                            hprojT[g][:, m, n * 512:(n + 1) * 512], ps,
                            bh[:, m:m + 1])

        def bcast_t(ap2):
            # [128, GB(b)] -> [128, T(t, stride0), GB(b)]
            return _b.AP(tensor=ap2.tensor, offset=ap2.offset,
                         ap=[ap2.ap[0], [0, T], ap2.ap[1]])

        gate_sl = {"f": 1, "i": 0, "g": 2, "o": 3}
        # gate -> (psum tag, row offset): f/i share FI bank, g/o share GO bank
        gate_loc = {"f": ("FI", 0), "i": ("FI", 64), "g": ("GO", 0), "o": ("GO", 64)}

        def emit_hT_transpose(s):
            # h_bf [64, 512] -> hT [128, 4, 64] via 4 PE transposes
            phT = ptp.tile([128, 256], bf16, tag="tp", name=f"phT_{s}")
            for m in range(4):
                nc.tensor.transpose(phT[:, m * 64:(m + 1) * 64],
                                    hbf[0][:, m * 128:(m + 1) * 128],
                                    ident[0:BS, 0:BS])
            hT[0] = state.tile([128, 4, BS], bf16, tag="hT", name=f"hT_{s}")
            nc.vector.tensor_copy(hT[0], phT)

        def emit_qT(s):
            # qT[h',b] = sum_h Wh[h,h'] hT[h,b]; m-outer so chunk m is
            # copied out as soon as its k-accumulation finishes.
            pqT = psm.tile([128, 256], f32, tag="pq", name=f"pqT_{s}")
            qT = small.tile([128, 4, BS], bf16, tag="qT", bufs=2, name=f"qT_{s}")
            for m in range(4):
                for k in range(4):
                    nc.tensor.matmul(pqT[:, m * 64:(m + 1) * 64],
                                     wh[:, k, m * 128:(m + 1) * 128],
                                     hT[0][:, k, :],
                                     start=(k == 0), stop=(k == 3))
                nc.vector.tensor_copy(qT[:, m, :], pqT[:, m * 64:(m + 1) * 64])
            return qT

        def emit_probs(sm1):
            # probs(sm1) = h(sm1) @ Wgen + bg, from hT
            pp = psm.tile([128, 256], f32, tag="pq", name=f"pp_{sm1}")
            for k in range(4):
                nc.tensor.matmul(pp[0:BS, 0:NCC], hT[0][:, k, :], wg[:, k, :],
                                 start=(k == 0), stop=(k == 3))
            nc.vector.tensor_add(pr_all[:, sm1, :], pp[0:BS, 0:NCC], bg)

        def emit_z_early(s, pzt):
            # onehot@Ko' (start) + h@R during the tanh window
            for gn in "figo":
                tag, ro = gate_loc[gn]
                zsl = slice(gate_sl[gn] * 512, (gate_sl[gn] + 1) * 512)
                nc.tensor.matmul(pzt[tag][ro:ro + 64, :], oh[:, s, :],
                                 ko[:, zsl], start=True, stop=False,
                                 tile_position=(0, ro))
            if s == 0:
                return  # h == 0: R contributes nothing
            for k in range(4):
                for gn in "figo":
                    tag, ro = gate_loc[gn]
                    zsl = slice(gate_sl[gn] * 512, (gate_sl[gn] + 1) * 512)
                    nc.tensor.matmul(pzt[tag][ro:ro + 64, :], hT[0][:, k, :],
                                     rr[:, k, zsl], start=False, stop=False,
                                     tile_position=(0, ro))

        def emit_att_tanh(s, g, qT, pe_):
            # DVE add + ACT tanh + e quadrant mms for group g
            gsl_b = slice(g * GB, (g + 1) * GB)
            th = big.tile([128, 4, T * GB], bf16, tag=f"th{g}", name=f"th{g}_{s}")
            for k in range(4):
                if qT is None:
                    # step 0: q == 0, tanh straight off hprojT
                    nc.scalar.activation(out=th[:, k, :],
                                         in_=hprojT[g][:, k, :], func=AF.Tanh)
                else:
                    nc.vector.tensor_add(
                        th[:, k, :].rearrange("p (t b) -> p t b", t=T),
                        hprojT[g][:, k, :].rearrange("p (t b) -> p t b", t=T),
                        bcast_t(qT[:, k, gsl_b]))
                    nc.scalar.activation(out=th[:, k, :], in_=th[:, k, :],
                                         func=AF.Tanh)
                for j in range(NCH):
                    bp = 32 * j
                    nc.tensor.matmul(pe_[bp:bp + 32, :], ws[:, k, :],
                                     th[:, k, j * 512:(j + 1) * 512],
                                     start=(k == 0), stop=(k == 3),
                                     tile_position=(0, bp))

        def emit_exp_scatter(s, g, pe_):
            # exp on the PSUM layout: est[32j, tl*32+b] = ex(t=16j+tl, b).
            # ablk (block-diag) is written DIRECTLY from est (2 DMAs, one
            # per b-parity); alphaT (only feeds the denominator matmul)
            # via one merged DMA on the gpsimd queue.
            est = small.tile([128, 512], bf16, tag=f"est{g}", bufs=1,
                             name=f"est{g}_{s}")
            nc.scalar.activation(out=est, in_=pe_, func=AF.Exp)
            ea = est[:]
            pp = ea.ap[0][0]
            alphaT = small.tile([T, GB], bf16, tag=f"alphaT{g}", bufs=2,
                                name=f"alphaT{g}_{s}")
            at = alphaT[:]
            for j in range(4):
                esl = est[32 * j:32 * j + 1, :]
                srcj = _b.AP(tensor=esl.tensor, offset=esl.offset,
                             ap=[[esl.ap[0][0], 1], [GB, T // 4], [1, GB]])
                eng = nc.gpsimd if j % 2 == 0 else dma
                eng.dma_start(out=alphaT[16 * j:16 * (j + 1), :], in_=srcj)
            # ablk block-diag scatter (2 DMAs, sync queue)
            ab = ablk[g][:]
            for par in (0, 1):
                srcp = _b.AP(tensor=at.tensor, offset=at.offset + par * at.ap[1][0],
                             ap=[[at.ap[0][0], T], [2 * at.ap[1][0], GB // 2]])
                dst = _b.AP(tensor=ab.tensor,
                            offset=ab.offset + par * (64 * ab.ap[0][0] + ab.ap[2][0]),
                            ap=[[ab.ap[0][0], T], [ab.ap[1][0] + 2 * ab.ap[2][0], GB // 2]])
                dma.dma_start(out=dst, in_=srcp)
            return alphaT

        def emit_post(s, g, ctx_sb, alphaT, psums, pctx, rcp_t):
            # denominator: sums[b] = alphaT^T @ ones (one matmul, N=1);
            # group g occupies psum rows g*GB..(g+1)*GB via array-col
            # quadrant (0, g*GB) so the two groups' ctx streams overlap
            ro = g * GB
            nc.tensor.matmul(psums[ro:ro + GB, 0:1], alphaT, ones,
                             start=True, stop=True, tile_position=(0, ro))
            nc.vector.reciprocal(rcp_t[ro:ro + GB, :], psums[ro:ro + GB, 0:1])
            for kt in range(GB // 2):
                nc.tensor.matmul(pctx[ro:ro + GB, :], ablk[g][:, kt, :],
                                 bHc[g][:, kt, :],
                                 start=(kt == 0), stop=(kt == GB // 2 - 1),
                                 tile_position=(0, ro))
            nc.vector.tensor_scalar_mul(ctx_sb[ro:ro + GB, :],
                                        pctx[ro:ro + GB, :],
                                        rcp_t[ro:ro + GB, :])

        def emit_ctxT(s, g, ctx_sb, pxT, xTc):
            # ctx rows of group g -> xTc[:, k, g*32:(g+1)*32]
            ro = g * GB
            for k in range(4):
                nc.tensor.transpose(pxT[g][:, k * GB:(k + 1) * GB],
                                    ctx_sb[ro:ro + GB, k * 128:(k + 1) * 128],
                                    ident[ro:ro + GB, ro:ro + GB])
            src = pxT[g][:, 0:128].rearrange("p (k b) -> p k b", k=4)
            nc.vector.tensor_copy(xTc[:, :, g * GB:(g + 1) * GB], src)

        def emit_z_late(s, pzt, xTc):
            for pair in ("fi", "go"):
                for k in range(4):
                    for gn in pair:
                        tag, ro = gate_loc[gn]
                        zsl = slice(gate_sl[gn] * 512, (gate_sl[gn] + 1) * 512)
                        nc.tensor.matmul(pzt[tag][ro:ro + 64, :], xTc[:, k, :],
                                         kc[:, k, zsl], start=False,
                                         stop=(k == 3), tile_position=(0, ro))

        def emit_gates(s, pzt):
            sig = {}
            t1 = t2 = None
            for gn in ("f", "i", "g", "o"):
                tag, ro = gate_loc[gn]
                g_sb = gates.tile([BS, H], f32, tag="gate", bufs=4)
                if gn == "g":
                    nc.scalar.activation(out=g_sb, in_=pzt[tag][ro:ro + 64, :],
                                         func=AF.Tanh)
                else:
                    # sigmoid via tanh: keeps ACT on the exp_and_others table
                    nc.scalar.activation(out=g_sb, in_=pzt[tag][ro:ro + 64, :],
                                         func=AF.Tanh, scale=0.5)
                    nc.vector.tensor_scalar(out=g_sb, in0=g_sb,
                                            scalar1=0.5, scalar2=0.5,
                                            op0=mybir.AluOpType.mult,
                                            op1=mybir.AluOpType.add)
                sig[gn] = g_sb
                if gn == "f":
                    t1 = gates.tile([BS, H], f32, tag="tmp", bufs=2)
                    nc.vector.tensor_mul(t1, sig["f"], c_st[0])
                elif gn == "g":
                    t2 = gates.tile([BS, H], f32, tag="tmp", bufs=2)
                    nc.vector.tensor_mul(t2, sig["i"], sig["g"])
                    c_st[0] = state.tile([BS, H], f32, tag="c", name=f"c_{s}")
                    nc.vector.tensor_add(c_st[0], t1, t2)
            tc_sb = gates.tile([BS, H], f32, tag="tmp", bufs=2)
            nc.scalar.activation(out=tc_sb, in_=c_st[0], func=AF.Tanh)
            hbf[0] = small.tile([BS, H], bf16, tag="h_bf", bufs=2,
                                name=f"h_bf_{s}")
            nc.vector.tensor_mul(hbf[0], sig["o"], tc_sb)

        for s in range(S):
            if s > 0:
                emit_hT_transpose(s)
                qT = emit_qT(s)
                emit_probs(s - 1)
            else:
                qT = None
            pzt = {"FI": pz.tile([128, 512], f32, tag="FI", name=f"pzFI_{s}"),
                   "GO": pz.tile([128, 512], f32, tag="GO", name=f"pzGO_{s}")}
            emit_z_early(s, pzt)
            pe_ = [None] * NG
            ctx_sb = small.tile([BS, C], bf16, tag="ctx_sb", bufs=2,
                                name=f"ctx_{s}")
            psums = psm.tile([128, 256], f32, tag="pq", name=f"psm_{s}")
            pctx = pcp.tile([128, C], f32, tag="ctx", name=f"pctx_{s}")
            rcp_t = tiny.tile([BS, 1], f32, tag="rcp")
            pxT = [None] * NG
            xTc = small.tile([128, 4, BS], bf16, tag="xTc", bufs=2,
                             name=f"xTc_{s}")
            aTs = [None] * NG
            for g in range(NG):
                pe_[g] = pep.tile([128, 512], f32, tag="pe", name=f"pe{g}_{s}")
                emit_att_tanh(s, g, qT, pe_[g])
                aTs[g] = emit_exp_scatter(s, g, pe_[g])
            for g in range(NG):
                emit_post(s, g, ctx_sb, aTs[g], psums, pctx, rcp_t)
                pxT[g] = ptp.tile([128, 256], bf16, tag="tp", name=f"pxT{g}_{s}")
                emit_ctxT(s, g, ctx_sb, pxT, xTc)
            emit_z_late(s, pzt, xTc)
            emit_gates(s, pzt)
        emit_hT_transpose(S)
        emit_probs(S - 1)
        dma.dma_start(out=out_d[:], in_=pr_all)

    nc.finalize()
    return nc


def _prep_core(inputs, i):
    bsl = slice(i * BS, (i + 1) * BS)
    bh_i = np.asarray(inputs["batch_H"][bsl], np.float32)          # [64, 64, 512]
    text_i = np.asarray(inputs["text"][bsl])                       # [64, 26]
    bh_g = bh_i.reshape(NG, GB, T, C)
    m = {}
    m["bHT"] = np.ascontiguousarray(bh_g.transpose(0, 3, 2, 1)).astype(BF)
    m["bHc"] = np.ascontiguousarray(bh_g.reshape(NG, GB // 2, 128, C)).astype(BF)
    m["wi"] = np.asarray(inputs["Wi"], np.float32).astype(BF)
    m["wh"] = np.asarray(inputs["Wh"], np.float32).astype(BF)
    m["bh"] = np.ascontiguousarray(
        np.asarray(inputs["bh"], np.float32).reshape(4, 128).T)
    wsr = np.ascontiguousarray(
        np.asarray(inputs["Ws"], np.float32)[:, 0].reshape(4, 128).T).astype(BF)
    m["ws"] = np.repeat(wsr[:, :, None], 32, axis=2)
    lk = np.asarray(inputs["lstm_kernel"], np.float32)
    lb = np.asarray(inputs["lstm_bias"], np.float32)
    m["kc"] = lk[:C].astype(BF)
    m["ko"] = (lk[C:] + lb[None, :]).astype(BF)
    m["rr"] = np.asarray(inputs["lstm_rec"], np.float32).astype(BF)
    m["oh"] = (np.arange(NCC)[:, None, None] == text_i.T[None, :, :]).astype(BF)
    m["wg"] = np.asarray(inputs["Wgen"], np.float32).astype(BF)
    m["bg"] = np.tile(np.asarray(inputs["bgen"], np.float32)[None, :], (BS, 1))
    m["ident"] = np.eye(128, dtype=np.float32).astype(BF)
    return m


def kernel(_trace=False, **inputs):
    from concourse import bass_utils
    if "nc" not in _CACHE:
        _CACHE["nc"] = build_bass()
    nc = _CACHE["nc"]
    in_maps = [_prep_core(inputs, i) for i in range(NCORES)]
    res = bass_utils.run_bass_kernel_spmd(nc, in_maps, list(range(NCORES)),
                                          trace=_trace)
    _CACHE["last_result"] = res
    out = np.concatenate([r["out"] for r in res.results], axis=0)
    return out.astype(np.float32)
